# revision 1
# baseline (speedup 1.0000x reference)
"""Trainium2 Bass kernel for nn_DHSMoERBFDetector.

Reference math (B=8192, D=768, NC=4, R=128, E=20, H=1024):
    rbf[c,b,r] = exp(coeff[c] * (feats[c,b] - offset[c,r])^2)
    x = [emb | rbf-features]                      # [B, 1280]
    h_e = relu(x @ W1[e] + b1[e]); pred_e = h_e @ W2[e] + b2[e]
    out = concat_e(pred_e)[inv]  with inv = argsort(argsort(component_idx))

Key fact: inv has values < B, so only expert 0's predictions are ever
selected.  The output is exactly  (relu(x @ W1[0] + b1[0]) @ W2[0] + b2[0])[inv].

Strategy: data-parallel over batch, 1024 rows per core on 8 cores.
Each core computes x^T (K-major: contraction dim on partitions) in SBUF --
emb arrives host-transposed, RBF features are generated on-device already
K-major via a K=1 broadcast matmul + ScalarE Square/Exp -- then runs the
expert-0 MLP (K=1280 contraction in 10 chunks of 128; H=1024 as 8 chunks
of M=128; batch as N=512 moving operand) accumulating in PSUM, ReLU fused
with the b1 bias on ScalarE, and an M=1 matmul for the H->1 dot product.
The inverse permutation is an int gather of 8192 floats, done on host.

Matmul precision modes (KERNEL_MODE env var):
  bf16x3 (default): x and W1 split into bf16 value+residual pairs;
      x@W1 = xh@Wh + xh@Wl + xl@Wh -- 3 bf16 matmuls at 1 PE cycle/row
      each vs plain fp32's 4 cycles/row, fp32 PSUM accumulation.
      HW-measured end-to-end error 6.3e-7 relative (plain fp32: 6.6e-7).
  fp32: plain fp32 matmuls (4 cycles/row), bit-nearest the reference.
  f32r1: single-pass f32r matmuls (trn2's 11-bit-mantissa fp32 mode,
      1 cycle/row; ~2e-4 relative error) -- fastest, ~74us/core in the
      cost model vs bf16x3's 136us, if ~2e-4 error is acceptable.
  f32r3/f32rh: f32r-based 3-pass splits (~1e-7); f32rh hit an
      NRT_EXEC_UNIT_UNRECOVERABLE on hardware (mixed-dtype PSUM
      accumulation groups suspected) -- kept for reference only.
"""

import os

import numpy as np

import concourse.bacc as bacc
import concourse.bass as bass
import concourse.mybir as mybir
import concourse.tile as tile
from concourse.bass_utils import run_bass_kernel_spmd

FP32 = mybir.dt.float32
F32R = mybir.dt.float32r
BF16 = mybir.dt.bfloat16
AF = mybir.ActivationFunctionType

B, D, NCOL, R, E, H = 8192, 768, 4, 128, 20, 1024
KTOT = D + NCOL * R          # 1280 contraction dim
NCORES = 8
BL = B // NCORES             # 1024 batch rows per core
KC = KTOT // 128             # 10 k-chunks (0-5 emb, 6-9 rbf)
KC_EMB = D // 128            # 6
HC = H // 128                # 8 hidden chunks
NT = BL // 512               # 2 batch tiles of N=512


def _consts(nc, consts, dram_map):
    sb = {}
    sb["feats"] = consts.tile([1, NCOL * BL], FP32, tag="feats", name="feats_sb")
    sb["noff"] = consts.tile([R, NCOL], FP32, tag="noff", name="noff_sb")
    sb["coef"] = consts.tile([R, NCOL], FP32, tag="coef", name="coef_sb")
    sb["b1"] = consts.tile([128, HC], FP32, tag="b1", name="b1_sb")
    sb["b2"] = consts.tile([1, 1], FP32, tag="b2", name="b2_sb")
    sb["ones"] = consts.tile([1, 128], FP32, tag="ones", name="ones_sb")
    nc.vector.memset(sb["ones"], 1.0)
    for key, src in dram_map.items():
        nc.sync.dma_start(sb[key], src[:, :])
    return sb


def _rbf_psum(nc, pp, sb, c, n):
    """Broadcast feats[c] across partitions into a PSUM tile via K=1 matmul."""
    bc = pp.tile([128, 512], FP32, tag="ps", name=f"bc_{c}_{n}")
    nc.tensor.matmul(
        bc, lhsT=sb["ones"][:, :],
        rhs=sb["feats"][:, c * BL + n * 512 : c * BL + (n + 1) * 512],
        start=True, stop=True,
    )
    return bc


def _build_f32r(nc, tc, dram, pools, three_pass: bool):
    """f32r matmul pipeline; three_pass adds the two residual terms.

    three_pass keeps SBUF under budget by streaming feats tiles, keeping h
    chunks per-n-tile only ([128,512] per m, reused across n), and using a
    plain fp32 second matmul (no h split needed).
    """
    big, consts, tmp, outp, pp = pools
    d = dram
    sb = {}
    sb["noff"] = consts.tile([R, NCOL], FP32, tag="noff", name="noff_sb")
    sb["coef"] = consts.tile([R, NCOL], FP32, tag="coef", name="coef_sb")
    sb["b1"] = consts.tile([128, HC], FP32, tag="b1", name="b1_sb")
    sb["b2"] = consts.tile([1, 1], FP32, tag="b2", name="b2_sb")
    sb["ones"] = consts.tile([1, 128], FP32, tag="ones", name="ones_sb")
    nc.vector.memset(sb["ones"], 1.0)
    for key, src in [("noff", d["noff"]), ("coef", d["coef"]),
                     ("b1", d["b1c"]), ("b2", d["b2c"])]:
        nc.sync.dma_start(sb[key], src[:, :])
    w2r_sb = consts.tile([128, HC], F32R, tag="w2r", name="w2r_sb")
    nc.sync.dma_start(w2r_sb, d["w2r"][:, :])
    w2f_sb = None
    if three_pass:
        w2f_sb = consts.tile([128, HC], FP32, tag="w2f", name="w2f_sb")
        nc.sync.dma_start(w2f_sb, d["w2c"][:, :])

    xr = [big.tile([128, BL], F32R, tag=f"xr{k}", name=f"xr{k}")
          for k in range(KC)]
    wr = [big.tile([128, H], F32R, tag=f"wr{k}", name=f"wr{k}")
          for k in range(KC)]
    if three_pass:
        xl = [big.tile([128, BL], F32R, tag=f"xl{k}", name=f"xl{k}")
              for k in range(KC)]
        wl = [big.tile([128, H], F32R, tag=f"wl{k}", name=f"wl{k}")
              for k in range(KC)]

    # RBF features first: their small feats DMAs must not queue behind the
    # big weight DMAs (the in-order PE's first instruction waits on them).
    for c in range(NCOL):
        for n in range(NT):
            bsl = slice(n * 512, (n + 1) * 512)
            kk = KC_EMB + c
            fe = tmp.tile([1, 512], FP32, tag="fe")
            nc.sync.dma_start(
                fe, d["feats"][:, c * BL + n * 512 : c * BL + (n + 1) * 512])
            bc = pp.tile([128, 512], FP32, tag="ps", name=f"bc_{c}_{n}")
            nc.tensor.matmul(bc, lhsT=sb["ones"][:, :], rhs=fe,
                             start=True, stop=True)
            d2 = tmp.tile([128, 512], FP32, tag="d2")
            nc.scalar.activation(d2, bc, AF.Square,
                                 bias=sb["noff"][:, c : c + 1], scale=1.0)
            if three_pass:
                rb = tmp.tile([128, 512], FP32, tag="rb")
                nc.scalar.activation(rb, d2, AF.Exp,
                                     scale=sb["coef"][:, c : c + 1])
                nc.vector.tensor_copy(xr[kk][:, bsl], rb)   # round to f32r
                nc.vector.tensor_tensor(
                    xl[kk][:, bsl], rb, xr[kk][:, bsl].bitcast(FP32),
                    mybir.AluOpType.subtract,
                )                                           # residual, rounded
            else:
                nc.scalar.activation(xr[kk][:, bsl], d2, AF.Exp,
                                     scale=sb["coef"][:, c : c + 1])

    for k in range(KC):
        ksl = slice(k * 128, (k + 1) * 128)
        nc.sync.dma_start(wr[k][:, :], d["w1r"][ksl, :])
        if three_pass:
            nc.sync.dma_start(wl[k][:, :], d["w1l"][ksl, :])
        if k < KC_EMB:
            nc.sync.dma_start(xr[k][:, :], d["ehr"][ksl, :])
            if three_pass:
                nc.sync.dma_start(xl[k][:, :], d["ehl"][ksl, :])

    # h^T = relu(W1^T x + b1); f32r1 writes relu straight to f32r tiles
    h_dt = FP32 if three_pass else F32R
    h_len = 512 if three_pass else BL
    hs = [big.tile([128, h_len], h_dt, tag=f"h{m}", name=f"h{m}")
          for m in range(HC)]
    for n in range(NT):
        bsl = slice(n * 512, (n + 1) * 512)
        hsl = slice(0, 512) if three_pass else bsl
        # pred accumulates per group so the last group's relu drain overlaps
        # with the earlier groups' pred matmuls
        p2 = pp.tile([1, 512], FP32, tag="ps", name=f"p2_{n}")
        w2 = w2f_sb if three_pass else w2r_sb
        for g in range(2):
            ms = range(4 * g, 4 * g + 4)
            ps = {m: pp.tile([128, 512], FP32, tag="ps", name=f"ps_{n}_{g}_{m}")
                  for m in ms}
            for k in range(KC):
                for m in ms:
                    msl = slice(m * 128, (m + 1) * 128)
                    first, last = k == 0, k == KC - 1
                    if three_pass:
                        nc.tensor.matmul(ps[m], lhsT=wr[k][:, msl],
                                         rhs=xr[k][:, bsl],
                                         start=first, stop=False)
                        nc.tensor.matmul(ps[m], lhsT=wr[k][:, msl],
                                         rhs=xl[k][:, bsl],
                                         start=False, stop=False)
                        nc.tensor.matmul(ps[m], lhsT=wl[k][:, msl],
                                         rhs=xr[k][:, bsl],
                                         start=False, stop=last)
                    else:
                        nc.tensor.matmul(ps[m], lhsT=wr[k][:, msl],
                                         rhs=xr[k][:, bsl],
                                         start=first, stop=last)
            for m in ms:
                nc.scalar.activation(hs[m][:, hsl], ps[m], AF.Relu,
                                     bias=sb["b1"][:, m : m + 1], scale=1.0)
            for m in ms:
                nc.tensor.matmul(p2, lhsT=w2[:, m : m + 1], rhs=hs[m][:, hsl],
                                 start=(m == 0), stop=(m == HC - 1))
        o_sb = outp.tile([1, 512], FP32, tag="o")
        nc.vector.tensor_scalar_add(o_sb, p2, sb["b2"][:1, :1])
        nc.sync.dma_start(d["out"][:, bsl], o_sb)


def _build_f32rh(nc, tc, dram, pools):
    """Hybrid: main term in f32r (11-bit, 1 cycle/row), correction terms in
    bf16.  x@W = xr@wr + xh@wl + xl@wh with xr = f32r(x), xl = bf16(x - xr),
    xh = bf16(x) (same for W).  Error ~5e-7 relative -- fp32-grade -- at the
    same 3-cycles/row PE cost as bf16x3.

    SBUF budget forces: streamed feats tiles, per-n h chunks, bf16 "high"
    planes derived on-device from the f32r planes (zero extra DMA for them).
    """
    big, consts, tmp, outp, pp = pools
    d = dram
    sb = {}
    sb["noff"] = consts.tile([R, NCOL], FP32, tag="noff", name="noff_sb")
    sb["coef"] = consts.tile([R, NCOL], FP32, tag="coef", name="coef_sb")
    sb["b1"] = consts.tile([128, HC], FP32, tag="b1", name="b1_sb")
    sb["b2"] = consts.tile([1, 1], FP32, tag="b2", name="b2_sb")
    sb["ones"] = consts.tile([1, 128], FP32, tag="ones", name="ones_sb")
    nc.vector.memset(sb["ones"], 1.0)
    for key, src in [("noff", d["noff"]), ("coef", d["coef"]),
                     ("b1", d["b1c"]), ("b2", d["b2c"])]:
        nc.sync.dma_start(sb[key], src[:, :])
    w2f_sb = consts.tile([128, HC], FP32, tag="w2f", name="w2f_sb")
    nc.sync.dma_start(w2f_sb, d["w2c"][:, :])

    xr = [big.tile([128, BL], F32R, tag=f"xr{k}", name=f"xr{k}")
          for k in range(KC)]
    xh = [big.tile([128, BL], BF16, tag=f"xh{k}", name=f"xh{k}")
          for k in range(KC)]
    xl = [big.tile([128, BL], BF16, tag=f"xl{k}", name=f"xl{k}")
          for k in range(KC)]
    wr = [big.tile([128, H], F32R, tag=f"wr{k}", name=f"wr{k}")
          for k in range(KC)]
    wh = [big.tile([128, H], BF16, tag=f"wh{k}", name=f"wh{k}")
          for k in range(KC)]
    wl = [big.tile([128, H], BF16, tag=f"wl{k}", name=f"wl{k}")
          for k in range(KC)]

    # RBF features first (small feats DMAs must beat the big DMAs into the
    # queues; the in-order PE's first instruction waits on them)
    for c in range(NCOL):
        for n in range(NT):
            bsl = slice(n * 512, (n + 1) * 512)
            kk = KC_EMB + c
            fe = tmp.tile([1, 512], FP32, tag="fe")
            nc.sync.dma_start(
                fe, d["feats"][:, c * BL + n * 512 : c * BL + (n + 1) * 512])
            bc = pp.tile([128, 512], FP32, tag="ps", name=f"bc_{c}_{n}")
            nc.tensor.matmul(bc, lhsT=sb["ones"][:, :], rhs=fe,
                             start=True, stop=True)
            d2 = tmp.tile([128, 512], FP32, tag="d2")
            nc.scalar.activation(d2, bc, AF.Square,
                                 bias=sb["noff"][:, c : c + 1], scale=1.0)
            rb = tmp.tile([128, 512], FP32, tag="rb")
            nc.scalar.activation(rb, d2, AF.Exp,
                                 scale=sb["coef"][:, c : c + 1])
            nc.vector.tensor_copy(xr[kk][:, bsl], rb)       # round to f32r
            nc.vector.tensor_copy(xh[kk][:, bsl], rb)       # round to bf16
            back = tmp.tile([128, 512], FP32, tag="back")
            nc.vector.tensor_sub(back, rb, xr[kk][:, bsl].bitcast(FP32))
            nc.vector.tensor_copy(xl[kk][:, bsl], back)     # residual -> bf16

    # big DMAs (k-ascending so the first k-sweep streams) + derived bf16
    # "high" planes (DVE casts of the f32r planes; the 2^-12 difference vs
    # bf16(original) only enters the ~2^-13-scale correction terms)
    for k in range(KC):
        ksl = slice(k * 128, (k + 1) * 128)
        nc.sync.dma_start(wr[k][:, :], d["w1r"][ksl, :])
        nc.sync.dma_start(wl[k][:, :], d["w1lb"][ksl, :])
        if k < KC_EMB:
            nc.sync.dma_start(xr[k][:, :], d["ehr"][ksl, :])
            nc.sync.dma_start(xl[k][:, :], d["ehlb"][ksl, :])
            nc.vector.tensor_copy(xh[k][:, :], xr[k].bitcast(FP32))
        nc.vector.tensor_copy(wh[k][:, :], wr[k].bitcast(FP32))

    hs = [big.tile([128, 512], FP32, tag=f"h{m}", name=f"h{m}")
          for m in range(HC)]
    for n in range(NT):
        bsl = slice(n * 512, (n + 1) * 512)
        hsl = slice(0, 512)
        p2 = pp.tile([1, 512], FP32, tag="ps", name=f"p2_{n}")
        for g in range(2):
            ms = range(4 * g, 4 * g + 4)
            ps = {m: pp.tile([128, 512], FP32, tag="ps", name=f"ps_{n}_{g}_{m}")
                  for m in ms}
            for k in range(KC):
                for m in ms:
                    msl = slice(m * 128, (m + 1) * 128)
                    nc.tensor.matmul(ps[m], lhsT=wr[k][:, msl],
                                     rhs=xr[k][:, bsl],
                                     start=(k == 0), stop=False)
                    nc.tensor.matmul(ps[m], lhsT=wh[k][:, msl],
                                     rhs=xl[k][:, bsl],
                                     start=False, stop=False)
                    nc.tensor.matmul(ps[m], lhsT=wl[k][:, msl],
                                     rhs=xh[k][:, bsl],
                                     start=False, stop=(k == KC - 1))
            for m in ms:
                nc.scalar.activation(hs[m][:, hsl], ps[m], AF.Relu,
                                     bias=sb["b1"][:, m : m + 1], scale=1.0)
            for m in ms:
                nc.tensor.matmul(p2, lhsT=w2f_sb[:, m : m + 1],
                                 rhs=hs[m][:, hsl],
                                 start=(m == 0), stop=(m == HC - 1))
        o_sb = outp.tile([1, 512], FP32, tag="o")
        nc.vector.tensor_scalar_add(o_sb, p2, sb["b2"][:1, :1])
        nc.sync.dma_start(d["out"][:, bsl], o_sb)


def _build_fp32(nc, tc, dram, pools):
    big, consts, tmp, outp, pp = pools
    d = dram
    sb = _consts(nc, consts, dict(
        feats=d["feats"], noff=d["noff"], coef=d["coef"],
        b1=d["b1c"], b2=d["b2c"],
    ))
    w2_sb = consts.tile([128, HC], FP32, tag="w2")
    nc.sync.dma_start(w2_sb, d["w2c"][:, :])

    xt = [big.tile([128, BL], FP32, tag=f"xt{k}", name=f"xt{k}")
          for k in range(KC)]
    w1s = [big.tile([128, H], FP32, tag=f"w1_{k}", name=f"w1_{k}")
           for k in range(KC)]
    hs = [big.tile([128, BL], FP32, tag=f"h{m}", name=f"h{m}")
          for m in range(HC)]

    for k in range(KC):
        nc.sync.dma_start(w1s[k][:, :], d["w1"][k * 128 : (k + 1) * 128, :])
        if k < KC_EMB:
            nc.sync.dma_start(xt[k][:, :], d["embT"][k * 128 : (k + 1) * 128, :])

    for c in range(NCOL):
        for n in range(NT):
            bsl = slice(n * 512, (n + 1) * 512)
            bc = _rbf_psum(nc, pp, sb, c, n)
            d2 = tmp.tile([128, 512], FP32, tag="d2")
            nc.scalar.activation(d2, bc, AF.Square,
                                 bias=sb["noff"][:, c : c + 1], scale=1.0)
            nc.scalar.activation(xt[KC_EMB + c][:, bsl], d2, AF.Exp,
                                 scale=sb["coef"][:, c : c + 1])

    for n in range(NT):
        bsl = slice(n * 512, (n + 1) * 512)
        for g in range(2):
            ms = range(4 * g, 4 * g + 4)
            ps = {m: pp.tile([128, 512], FP32, tag="ps", name=f"ps_{n}_{g}_{m}")
                  for m in ms}
            for k in range(KC):
                for m in ms:
                    nc.tensor.matmul(
                        ps[m], lhsT=w1s[k][:, m * 128 : (m + 1) * 128],
                        rhs=xt[k][:, bsl],
                        start=(k == 0), stop=(k == KC - 1),
                    )
            for m in ms:
                nc.scalar.activation(hs[m][:, bsl], ps[m], AF.Relu,
                                     bias=sb["b1"][:, m : m + 1], scale=1.0)

    for n in range(NT):
        bsl = slice(n * 512, (n + 1) * 512)
        p2 = pp.tile([1, 512], FP32, tag="ps", name=f"p2_{n}")
        for m in range(HC):
            nc.tensor.matmul(p2, lhsT=w2_sb[:, m : m + 1], rhs=hs[m][:, bsl],
                             start=(m == 0), stop=(m == HC - 1))
        o_sb = outp.tile([1, 512], FP32, tag="o")
        nc.vector.tensor_scalar_add(o_sb, p2, sb["b2"][:1, :1])
        nc.sync.dma_start(d["out"][:, bsl], o_sb)


def _build_bf16x3(nc, tc, dram, pools):
    big, consts, tmp, outp, pp = pools
    d = dram
    sb = _consts(nc, consts, dict(
        feats=d["feats"], noff=d["noff"], coef=d["coef"],
        b1=d["b1c"], b2=d["b2c"],
    ))
    w2_sb = consts.tile([128, HC], FP32, tag="w2")
    nc.sync.dma_start(w2_sb, d["w2c"][:, :])

    xh = [big.tile([128, BL], BF16, tag=f"xh{k}", name=f"xh{k}")
          for k in range(KC)]
    xl = [big.tile([128, BL], BF16, tag=f"xl{k}", name=f"xl{k}")
          for k in range(KC)]
    wh = [big.tile([128, H], BF16, tag=f"wh{k}", name=f"wh{k}")
          for k in range(KC)]
    wl = [big.tile([128, H], BF16, tag=f"wl{k}", name=f"wl{k}")
          for k in range(KC)]
    hs = [big.tile([128, BL], FP32, tag=f"h{m}", name=f"h{m}")
          for m in range(HC)]

    for k in range(KC):
        ksl = slice(k * 128, (k + 1) * 128)
        nc.sync.dma_start(wh[k][:, :], d["w1h"][ksl, :])
        nc.sync.dma_start(wl[k][:, :], d["w1l"][ksl, :])
        if k < KC_EMB:
            nc.sync.dma_start(xh[k][:, :], d["ehT"][ksl, :])
            nc.sync.dma_start(xl[k][:, :], d["elT"][ksl, :])

    for c in range(NCOL):
        for n in range(NT):
            bsl = slice(n * 512, (n + 1) * 512)
            kk = KC_EMB + c
            bc = _rbf_psum(nc, pp, sb, c, n)
            d2 = tmp.tile([128, 512], FP32, tag="d2")
            nc.scalar.activation(d2, bc, AF.Square,
                                 bias=sb["noff"][:, c : c + 1], scale=1.0)
            rb = tmp.tile([128, 512], FP32, tag="rb")
            nc.scalar.activation(rb, d2, AF.Exp,
                                 scale=sb["coef"][:, c : c + 1])
            nc.vector.tensor_copy(xh[kk][:, bsl], rb)      # round to bf16
            back = tmp.tile([128, 512], FP32, tag="back")
            nc.vector.tensor_copy(back, xh[kk][:, bsl])    # widen
            nc.vector.tensor_sub(back, rb, back)           # residual
            nc.vector.tensor_copy(xl[kk][:, bsl], back)

    for n in range(NT):
        bsl = slice(n * 512, (n + 1) * 512)
        p2 = pp.tile([1, 512], FP32, tag="ps", name=f"p2_{n}")
        for g in range(2):
            ms = range(4 * g, 4 * g + 4)
            ps = {m: pp.tile([128, 512], FP32, tag="ps", name=f"ps_{n}_{g}_{m}")
                  for m in ms}
            for k in range(KC):
                for m in ms:
                    msl = slice(m * 128, (m + 1) * 128)
                    nc.tensor.matmul(ps[m], lhsT=wh[k][:, msl],
                                     rhs=xh[k][:, bsl],
                                     start=(k == 0), stop=False)
                    nc.tensor.matmul(ps[m], lhsT=wh[k][:, msl],
                                     rhs=xl[k][:, bsl],
                                     start=False, stop=False)
                    nc.tensor.matmul(ps[m], lhsT=wl[k][:, msl],
                                     rhs=xh[k][:, bsl],
                                     start=False, stop=(k == KC - 1))
            for m in ms:
                nc.scalar.activation(hs[m][:, bsl], ps[m], AF.Relu,
                                     bias=sb["b1"][:, m : m + 1], scale=1.0)
            for m in ms:
                nc.tensor.matmul(p2, lhsT=w2_sb[:, m : m + 1],
                                 rhs=hs[m][:, bsl],
                                 start=(m == 0), stop=(m == HC - 1))
        o_sb = outp.tile([1, 512], FP32, tag="o")
        nc.vector.tensor_scalar_add(o_sb, p2, sb["b2"][:1, :1])
        nc.sync.dma_start(d["out"][:, bsl], o_sb)



def _build_bf16x3b(nc, tc, dram, pools):
    """bf16x3 + bf16-split second matmul (saves the 4-cycles/row fp32 PE
    cost of the H->1 dot: 16x853ns -> 48x213ns + overlapped DVE splits)."""
    big, consts, tmp, outp, pp = pools
    d = dram
    sb = _consts(nc, consts, dict(
        feats=d["feats"], noff=d["noff"], coef=d["coef"],
        b1=d["b1c"], b2=d["b2c"],
    ))
    w2h_sb = consts.tile([128, HC], BF16, tag="w2h", name="w2h_sb")
    w2l_sb = consts.tile([128, HC], BF16, tag="w2l", name="w2l_sb")
    nc.sync.dma_start(w2h_sb, d["w2h"][:, :])
    nc.sync.dma_start(w2l_sb, d["w2l"][:, :])

    xh = [big.tile([128, BL], BF16, tag=f"xh{k}", name=f"xh{k}")
          for k in range(KC)]
    xl = [big.tile([128, BL], BF16, tag=f"xl{k}", name=f"xl{k}")
          for k in range(KC)]
    wh = [big.tile([128, H], BF16, tag=f"wh{k}", name=f"wh{k}")
          for k in range(KC)]
    wl = [big.tile([128, H], BF16, tag=f"wl{k}", name=f"wl{k}")
          for k in range(KC)]
    hs = [big.tile([128, BL], FP32, tag=f"h{m}", name=f"h{m}")
          for m in range(HC)]
    hh = [big.tile([128, BL], BF16, tag=f"hh{m}", name=f"hh{m}")
          for m in range(HC)]
    hl = [big.tile([128, BL], BF16, tag=f"hl{m}", name=f"hl{m}")
          for m in range(HC)]

    for k in range(KC):
        ksl = slice(k * 128, (k + 1) * 128)
        nc.sync.dma_start(wh[k][:, :], d["w1h"][ksl, :])
        nc.sync.dma_start(wl[k][:, :], d["w1l"][ksl, :])
        if k < KC_EMB:
            nc.sync.dma_start(xh[k][:, :], d["ehT"][ksl, :])
            nc.sync.dma_start(xl[k][:, :], d["elT"][ksl, :])

    for c in range(NCOL):
        for n in range(NT):
            bsl = slice(n * 512, (n + 1) * 512)
            kk = KC_EMB + c
            bc = _rbf_psum(nc, pp, sb, c, n)
            d2 = tmp.tile([128, 512], FP32, tag="d2")
            nc.scalar.activation(d2, bc, AF.Square,
                                 bias=sb["noff"][:, c : c + 1], scale=1.0)
            rb = tmp.tile([128, 512], FP32, tag="rb")
            nc.scalar.activation(rb, d2, AF.Exp,
                                 scale=sb["coef"][:, c : c + 1])
            nc.vector.tensor_copy(xh[kk][:, bsl], rb)
            back = tmp.tile([128, 512], FP32, tag="back")
            nc.vector.tensor_copy(back, xh[kk][:, bsl])
            nc.vector.tensor_sub(back, rb, back)
            nc.vector.tensor_copy(xl[kk][:, bsl], back)

    for n in range(NT):
        bsl = slice(n * 512, (n + 1) * 512)
        p2 = pp.tile([1, 512], FP32, tag="ps", name=f"p2_{n}")
        for g in range(2):
            ms = range(4 * g, 4 * g + 4)
            ps = {m: pp.tile([128, 512], FP32, tag="ps", name=f"ps_{n}_{g}_{m}")
                  for m in ms}
            for k in range(KC):
                for m in ms:
                    msl = slice(m * 128, (m + 1) * 128)
                    nc.tensor.matmul(ps[m], lhsT=wh[k][:, msl],
                                     rhs=xh[k][:, bsl],
                                     start=(k == 0), stop=False)
                    nc.tensor.matmul(ps[m], lhsT=wh[k][:, msl],
                                     rhs=xl[k][:, bsl],
                                     start=False, stop=False)
                    nc.tensor.matmul(ps[m], lhsT=wl[k][:, msl],
                                     rhs=xh[k][:, bsl],
                                     start=False, stop=(k == KC - 1))
            for m in ms:
                nc.scalar.activation(hs[m][:, bsl], ps[m], AF.Relu,
                                     bias=sb["b1"][:, m : m + 1], scale=1.0)
                nc.vector.tensor_copy(hh[m][:, bsl], hs[m][:, bsl])
                back2 = tmp.tile([128, 512], FP32, tag="back2")
                nc.vector.tensor_copy(back2, hh[m][:, bsl])
                nc.vector.tensor_sub(back2, hs[m][:, bsl], back2)
                nc.vector.tensor_copy(hl[m][:, bsl], back2)
            for m in ms:
                mm = slice(m, m + 1)
                nc.tensor.matmul(p2, lhsT=w2h_sb[:, mm], rhs=hh[m][:, bsl],
                                 start=(m == 0), stop=False)
                nc.tensor.matmul(p2, lhsT=w2h_sb[:, mm], rhs=hl[m][:, bsl],
                                 start=False, stop=False)
                nc.tensor.matmul(p2, lhsT=w2l_sb[:, mm], rhs=hh[m][:, bsl],
                                 start=False, stop=(n >= 0 and m == HC - 1))
        o_sb = outp.tile([1, 512], FP32, tag="o")
        nc.vector.tensor_scalar_add(o_sb, p2, sb["b2"][:1, :1])
        nc.sync.dma_start(d["out"][:, bsl], o_sb)


def _build_nc(mode: str) -> bass.Bass:
    # Bacc (not raw Bass): its finalize() runs move_matmul_waits_to_ldweights
    # + generate_event_semaphores, which split semaphore waits that exceed
    # the per-instruction hardware limit (walrus otherwise fails codegen).
    nc = bacc.Bacc()

    d = {}
    d["feats"] = nc.dram_tensor("feats", [1, NCOL * BL], FP32,
                                kind="ExternalInput")
    d["b1c"] = nc.dram_tensor("b1c", [128, HC], FP32, kind="ExternalInput")
    d["w2c"] = nc.dram_tensor("w2c", [128, HC], FP32, kind="ExternalInput")
    d["b2c"] = nc.dram_tensor("b2c", [1, 1], FP32, kind="ExternalInput")
    d["noff"] = nc.dram_tensor("noff", [R, NCOL], FP32, kind="ExternalInput")
    d["coef"] = nc.dram_tensor("coef", [R, NCOL], FP32, kind="ExternalInput")
    d["out"] = nc.dram_tensor("out", [1, BL], FP32, kind="ExternalOutput")

    if mode == "fp32":
        d["embT"] = nc.dram_tensor("embT", [D, BL], FP32, kind="ExternalInput")
        d["w1"] = nc.dram_tensor("w1", [KTOT, H], FP32, kind="ExternalInput")
    elif mode in ("bf16x3", "bf16x3b"):
        for n2 in ("ehT", "elT"):
            d[n2] = nc.dram_tensor(n2, [D, BL], BF16, kind="ExternalInput")
        for n2 in ("w1h", "w1l"):
            d[n2] = nc.dram_tensor(n2, [KTOT, H], BF16, kind="ExternalInput")
        if mode == "bf16x3b":
            d["w2h"] = nc.dram_tensor("w2h", [128, HC], BF16,
                                      kind="ExternalInput")
            d["w2l"] = nc.dram_tensor("w2l", [128, HC], BF16,
                                      kind="ExternalInput")
    elif mode in ("f32r1", "f32r3"):
        d["ehr"] = nc.dram_tensor("ehr", [D, BL], F32R, kind="ExternalInput")
        d["w1r"] = nc.dram_tensor("w1r", [KTOT, H], F32R, kind="ExternalInput")
        d["w2r"] = nc.dram_tensor("w2r", [128, HC], F32R, kind="ExternalInput")
        if mode == "f32r3":
            d["ehl"] = nc.dram_tensor("ehl", [D, BL], F32R,
                                      kind="ExternalInput")
            d["w1l"] = nc.dram_tensor("w1l", [KTOT, H], F32R,
                                      kind="ExternalInput")
            d["w2l"] = nc.dram_tensor("w2l", [128, HC], F32R,
                                      kind="ExternalInput")
    elif mode == "f32rh":
        d["ehr"] = nc.dram_tensor("ehr", [D, BL], F32R, kind="ExternalInput")
        d["ehlb"] = nc.dram_tensor("ehlb", [D, BL], BF16, kind="ExternalInput")
        d["w1r"] = nc.dram_tensor("w1r", [KTOT, H], F32R, kind="ExternalInput")
        d["w1lb"] = nc.dram_tensor("w1lb", [KTOT, H], BF16,
                                   kind="ExternalInput")
    else:
        raise ValueError(mode)

    with tile.TileContext(nc) as tc:
        with (
            tc.tile_pool(name="big", bufs=1) as big,
            tc.tile_pool(name="consts", bufs=1) as consts,
            tc.tile_pool(name="tmp", bufs=3) as tmp,
            tc.tile_pool(name="outp", bufs=2) as outp,
            tc.tile_pool(name="psum", bufs=8, space="PSUM") as pp,
        ):
            pools = (big, consts, tmp, outp, pp)
            if mode == "fp32":
                _build_fp32(nc, tc, d, pools)
            elif mode == "bf16x3":
                _build_bf16x3(nc, tc, d, pools)
            elif mode == "bf16x3b":
                _build_bf16x3b(nc, tc, d, pools)
            elif mode == "f32rh":
                _build_f32rh(nc, tc, d, pools)
            else:
                _build_f32r(nc, tc, d, pools, three_pass=(mode == "f32r3"))

    # run Bacc's compile pipeline (wait splitting, register allocation);
    # run_bass_via_pjrt serializes nc.m as-is and never finalizes.
    nc.finalize()
    return nc


def _bf16_pair(a: np.ndarray):
    """Split fp32 array into (hi, lo) bf16 arrays with hi+lo ~ a."""
    import ml_dtypes

    hi = a.astype(ml_dtypes.bfloat16)
    lo = (a - hi.astype(np.float32)).astype(ml_dtypes.bfloat16)
    return hi, lo


def _round_f32r(a: np.ndarray) -> np.ndarray:
    """Round fp32 to f32r (11-bit mantissa, round-half-up at bit 12) --
    bit-exact with the hardware's cast (verified against gpsimd cast-DMA)."""
    v = np.ascontiguousarray(a, dtype=np.float32).view(np.uint32)
    r = (((v.astype(np.uint64) + (1 << 11)) >> 12) << 12).astype(np.uint32)
    return r.view(np.float32)


def _f32r_pair(a: np.ndarray):
    hi = _round_f32r(a)
    lo = _round_f32r(a - hi)
    return hi, lo


_NC_CACHE: dict = {}


def kernel(emb, feats, rbf_offset, rbf_coeff, W1, b1, W2, b2, component_idx):
    mode = os.environ.get("KERNEL_MODE", "f32r3")
    emb = np.ascontiguousarray(emb, dtype=np.float32)
    feats = np.ascontiguousarray(feats, dtype=np.float32)
    rbf_offset = np.asarray(rbf_offset, dtype=np.float32)
    rbf_coeff = np.asarray(rbf_coeff, dtype=np.float32)
    W1 = np.asarray(W1, dtype=np.float32)
    b1 = np.asarray(b1, dtype=np.float32)
    W2 = np.asarray(W2, dtype=np.float32)
    b2 = np.asarray(b2, dtype=np.float32)
    component_idx = np.asarray(component_idx)

    # shared (expert-0 only) tensors
    w1_full = np.ascontiguousarray(W1[0])                        # [1280, 1024]
    w2c = np.ascontiguousarray(W2[0, :, 0].reshape(HC, 128).T)   # [128, 8]
    shared = dict(
        b1c=np.ascontiguousarray(b1[0].reshape(HC, 128).T),      # [128, 8]
        w2c=w2c,
        b2c=b2[0].reshape(1, 1),
        noff=np.ascontiguousarray(-rbf_offset.T),                # [128, 4]
        coef=np.ascontiguousarray(
            np.broadcast_to(rbf_coeff[None, :], (R, NCOL))),     # [128, 4]
    )
    if mode == "fp32":
        shared["w1"] = w1_full
    elif mode in ("bf16x3", "bf16x3b"):
        shared["w1h"], shared["w1l"] = _bf16_pair(w1_full)
        if mode == "bf16x3b":
            shared["w2h"], shared["w2l"] = _bf16_pair(w2c)
    elif mode == "f32rh":
        import ml_dtypes

        shared["w1r"] = _round_f32r(w1_full)
        shared["w1lb"] = (w1_full - shared["w1r"]).astype(ml_dtypes.bfloat16)
    else:
        shared["w1r"], w1l = _f32r_pair(w1_full)
        w2r, w2l = _f32r_pair(w2c)
        shared["w2r"] = w2r
        if mode == "f32r3":
            shared["w1l"] = w1l
            shared["w2l"] = w2l

    in_maps = []
    for i in range(NCORES):
        s = slice(i * BL, (i + 1) * BL)
        m = dict(
            feats=np.ascontiguousarray(feats[:, s]).reshape(1, NCOL * BL),
            **shared,
        )
        embT = np.ascontiguousarray(emb[s].T)                    # [768, 1024]
        if mode == "fp32":
            m["embT"] = embT
        elif mode in ("bf16x3", "bf16x3b"):
            m["ehT"], m["elT"] = _bf16_pair(embT)
        elif mode == "f32rh":
            import ml_dtypes

            m["ehr"] = _round_f32r(embT)
            m["ehlb"] = (embT - m["ehr"]).astype(ml_dtypes.bfloat16)
        else:
            m["ehr"], ehl = _f32r_pair(embT)
            if mode == "f32r3":
                m["ehl"] = ehl
        in_maps.append(m)

    if mode not in _NC_CACHE:
        _NC_CACHE[mode] = _build_nc(mode)

    res = run_bass_kernel_spmd(_NC_CACHE[mode], in_maps, list(range(NCORES)))

    pred = np.concatenate(
        [res.results[i]["out"].reshape(BL) for i in range(NCORES)]
    )                                                            # [8192]

    order = np.argsort(component_idx, kind="stable")
    inv = np.argsort(order, kind="stable")
    return pred[inv].reshape(B, 1).astype(np.float32)



# revision 6
# speedup vs baseline: 1.4928x; 1.4928x over previous
"""Trainium2 Bass kernel for nn_DHSMoERBFDetector.

Reference math (B=8192, D=768, NC=4, R=128, E=20, H=1024):
    rbf[c,b,r] = exp(coeff[c] * (feats[c,b] - offset[c,r])^2)
    x = [emb | rbf-features]                      # [B, 1280]
    h_e = relu(x @ W1[e] + b1[e]); pred_e = h_e @ W2[e] + b2[e]
    out = concat_e(pred_e)[inv]  with inv = argsort(argsort(component_idx))

Key fact: inv has values < B, so only expert 0's predictions are ever
selected.  The output is exactly  (relu(x @ W1[0] + b1[0]) @ W2[0] + b2[0])[inv].

Strategy: data-parallel over batch, 1024 rows per core on 8 cores.
Each core computes x^T (K-major: contraction dim on partitions) in SBUF --
emb arrives host-transposed, RBF features are generated on-device already
K-major via a K=1 broadcast matmul + ScalarE Square/Exp -- then runs the
expert-0 MLP (K=1280 contraction in 10 chunks of 128; H=1024 as 8 chunks
of M=128; batch as N=512 moving operand) accumulating in PSUM, ReLU fused
with the b1 bias on ScalarE, and an M=1 matmul for the H->1 dot product.
The inverse permutation is an int gather of 8192 floats, done on host.

Matmul precision modes (KERNEL_MODE env var):
  bf16x3 (default): x and W1 split into bf16 value+residual pairs;
      x@W1 = xh@Wh + xh@Wl + xl@Wh -- 3 bf16 matmuls at 1 PE cycle/row
      each vs plain fp32's 4 cycles/row, fp32 PSUM accumulation.
      HW-measured end-to-end error 6.3e-7 relative (plain fp32: 6.6e-7).
  fp32: plain fp32 matmuls (4 cycles/row), bit-nearest the reference.
  f32r1: single-pass f32r matmuls (trn2's 11-bit-mantissa fp32 mode,
      1 cycle/row; ~2e-4 relative error) -- fastest, ~74us/core in the
      cost model vs bf16x3's 136us, if ~2e-4 error is acceptable.
  f32r3/f32rh: f32r-based 3-pass splits (~1e-7); f32rh hit an
      NRT_EXEC_UNIT_UNRECOVERABLE on hardware (mixed-dtype PSUM
      accumulation groups suspected) -- kept for reference only.
"""

import os

import numpy as np

import concourse.bacc as bacc
import concourse.bass as bass
import concourse.mybir as mybir
import concourse.tile as tile
from concourse.bass_utils import run_bass_kernel_spmd

FP32 = mybir.dt.float32
F32R = mybir.dt.float32r
BF16 = mybir.dt.bfloat16
AF = mybir.ActivationFunctionType

B, D, NCOL, R, E, H = 8192, 768, 4, 128, 20, 1024
KTOT = D + NCOL * R          # 1280 contraction dim
NCORES = 8
BL = B // NCORES             # 1024 batch rows per core
KC = KTOT // 128             # 10 k-chunks (0-5 emb, 6-9 rbf)
KC_EMB = D // 128            # 6
HC = H // 128                # 8 hidden chunks
NT = BL // 512               # 2 batch tiles of N=512


def _consts(nc, consts, dram_map):
    sb = {}
    sb["feats"] = consts.tile([1, NCOL * BL], FP32, tag="feats", name="feats_sb")
    sb["noff"] = consts.tile([R, NCOL], FP32, tag="noff", name="noff_sb")
    sb["coef"] = consts.tile([R, NCOL], FP32, tag="coef", name="coef_sb")
    sb["b1"] = consts.tile([128, HC], FP32, tag="b1", name="b1_sb")
    sb["b2"] = consts.tile([1, 1], FP32, tag="b2", name="b2_sb")
    sb["ones"] = consts.tile([1, 128], FP32, tag="ones", name="ones_sb")
    nc.vector.memset(sb["ones"], 1.0)
    for key, src in dram_map.items():
        nc.sync.dma_start(sb[key], src[:, :])
    return sb


def _rbf_psum(nc, pp, sb, c, n):
    """Broadcast feats[c] across partitions into a PSUM tile via K=1 matmul."""
    bc = pp.tile([128, 512], FP32, tag="ps", name=f"bc_{c}_{n}")
    nc.tensor.matmul(
        bc, lhsT=sb["ones"][:, :],
        rhs=sb["feats"][:, c * BL + n * 512 : c * BL + (n + 1) * 512],
        start=True, stop=True,
    )
    return bc


def _build_f32r(nc, tc, dram, pools, three_pass: bool):
    """f32r matmul pipeline; three_pass adds the two residual terms.

    three_pass keeps SBUF under budget by streaming feats tiles, keeping h
    chunks per-n-tile only ([128,512] per m, reused across n), and using a
    plain fp32 second matmul (no h split needed).
    """
    big, consts, tmp, outp, pp = pools
    d = dram
    sb = {}
    sb["noff"] = consts.tile([R, NCOL], FP32, tag="noff", name="noff_sb")
    sb["coef"] = consts.tile([R, NCOL], FP32, tag="coef", name="coef_sb")
    sb["b1"] = consts.tile([128, HC], FP32, tag="b1", name="b1_sb")
    sb["b2"] = consts.tile([1, 1], FP32, tag="b2", name="b2_sb")
    sb["ones"] = consts.tile([1, 128], FP32, tag="ones", name="ones_sb")
    nc.vector.memset(sb["ones"], 1.0)
    for key, src in [("noff", d["noff"]), ("coef", d["coef"]),
                     ("b1", d["b1c"]), ("b2", d["b2c"])]:
        nc.sync.dma_start(sb[key], src[:, :])
    w2r_sb = consts.tile([128, HC], F32R, tag="w2r", name="w2r_sb")
    nc.sync.dma_start(w2r_sb, d["w2r"][:, :])
    w2f_sb = None
    if three_pass:
        w2f_sb = consts.tile([128, HC], FP32, tag="w2f", name="w2f_sb")
        nc.sync.dma_start(w2f_sb, d["w2c"][:, :])

    xr = [big.tile([128, BL], F32R, tag=f"xr{k}", name=f"xr{k}")
          for k in range(KC)]
    wr = [big.tile([128, H], F32R, tag=f"wr{k}", name=f"wr{k}")
          for k in range(KC)]
    if three_pass:
        xl = [big.tile([128, BL], F32R, tag=f"xl{k}", name=f"xl{k}")
              for k in range(KC)]
        wl = [big.tile([128, H], F32R, tag=f"wl{k}", name=f"wl{k}")
              for k in range(KC)]

    # RBF features first: their small feats DMAs must not queue behind the
    # big weight DMAs (the in-order PE's first instruction waits on them).
    for c in range(NCOL):
        for n in range(NT):
            bsl = slice(n * 512, (n + 1) * 512)
            kk = KC_EMB + c
            fe = tmp.tile([1, 512], FP32, tag="fe")
            nc.sync.dma_start(
                fe, d["feats"][:, c * BL + n * 512 : c * BL + (n + 1) * 512])
            bc = pp.tile([128, 512], FP32, tag="ps", name=f"bc_{c}_{n}")
            nc.tensor.matmul(bc, lhsT=sb["ones"][:, :], rhs=fe,
                             start=True, stop=True)
            d2 = tmp.tile([128, 512], FP32, tag="d2")
            nc.scalar.activation(d2, bc, AF.Square,
                                 bias=sb["noff"][:, c : c + 1], scale=1.0)
            if three_pass:
                rb = tmp.tile([128, 512], FP32, tag="rb")
                nc.scalar.activation(rb, d2, AF.Exp,
                                     scale=sb["coef"][:, c : c + 1])
                nc.vector.tensor_copy(xr[kk][:, bsl], rb)   # round to f32r
                nc.vector.tensor_tensor(
                    xl[kk][:, bsl], rb, xr[kk][:, bsl].bitcast(FP32),
                    mybir.AluOpType.subtract,
                )                                           # residual, rounded
            else:
                nc.scalar.activation(xr[kk][:, bsl], d2, AF.Exp,
                                     scale=sb["coef"][:, c : c + 1])

    for k in range(KC):
        ksl = slice(k * 128, (k + 1) * 128)
        nc.sync.dma_start(wr[k][:, :], d["w1r"][ksl, :])
        if three_pass:
            nc.sync.dma_start(wl[k][:, :], d["w1l"][ksl, :])
        if k < KC_EMB:
            nc.sync.dma_start(xr[k][:, :], d["ehr"][ksl, :])
            if three_pass:
                nc.sync.dma_start(xl[k][:, :], d["ehl"][ksl, :])

    # h^T = relu(W1^T x + b1); f32r1 writes relu straight to f32r tiles
    h_dt = FP32 if three_pass else F32R
    h_len = 512 if three_pass else BL
    hs = [big.tile([128, h_len], h_dt, tag=f"h{m}", name=f"h{m}")
          for m in range(HC)]
    for n in range(NT):
        bsl = slice(n * 512, (n + 1) * 512)
        hsl = slice(0, 512) if three_pass else bsl
        # pred accumulates per group so the last group's relu drain overlaps
        # with the earlier groups' pred matmuls
        p2 = pp.tile([1, 512], FP32, tag="ps", name=f"p2_{n}")
        w2 = w2f_sb if three_pass else w2r_sb
        for g in range(2):
            ms = range(4 * g, 4 * g + 4)
            ps = {m: pp.tile([128, 512], FP32, tag="ps", name=f"ps_{n}_{g}_{m}")
                  for m in ms}
            for k in range(KC):
                for m in ms:
                    msl = slice(m * 128, (m + 1) * 128)
                    first, last = k == 0, k == KC - 1
                    if three_pass:
                        nc.tensor.matmul(ps[m], lhsT=wr[k][:, msl],
                                         rhs=xr[k][:, bsl],
                                         start=first, stop=False)
                        nc.tensor.matmul(ps[m], lhsT=wr[k][:, msl],
                                         rhs=xl[k][:, bsl],
                                         start=False, stop=False)
                        nc.tensor.matmul(ps[m], lhsT=wl[k][:, msl],
                                         rhs=xr[k][:, bsl],
                                         start=False, stop=last)
                    else:
                        nc.tensor.matmul(ps[m], lhsT=wr[k][:, msl],
                                         rhs=xr[k][:, bsl],
                                         start=first, stop=last)
            for m in ms:
                nc.scalar.activation(hs[m][:, hsl], ps[m], AF.Relu,
                                     bias=sb["b1"][:, m : m + 1], scale=1.0)
            for m in ms:
                nc.tensor.matmul(p2, lhsT=w2[:, m : m + 1], rhs=hs[m][:, hsl],
                                 start=(m == 0), stop=(m == HC - 1))
        o_sb = outp.tile([1, 512], FP32, tag="o")
        nc.vector.tensor_scalar_add(o_sb, p2, sb["b2"][:1, :1])
        nc.sync.dma_start(d["out"][:, bsl], o_sb)


def _build_f32rh(nc, tc, dram, pools):
    """Hybrid: main term in f32r (11-bit, 1 cycle/row), correction terms in
    bf16.  x@W = xr@wr + xh@wl + xl@wh with xr = f32r(x), xl = bf16(x - xr),
    xh = bf16(x) (same for W).  Error ~5e-7 relative -- fp32-grade -- at the
    same 3-cycles/row PE cost as bf16x3.

    SBUF budget forces: streamed feats tiles, per-n h chunks, bf16 "high"
    planes derived on-device from the f32r planes (zero extra DMA for them).
    """
    big, consts, tmp, outp, pp = pools
    d = dram
    sb = {}
    sb["noff"] = consts.tile([R, NCOL], FP32, tag="noff", name="noff_sb")
    sb["coef"] = consts.tile([R, NCOL], FP32, tag="coef", name="coef_sb")
    sb["b1"] = consts.tile([128, HC], FP32, tag="b1", name="b1_sb")
    sb["b2"] = consts.tile([1, 1], FP32, tag="b2", name="b2_sb")
    sb["ones"] = consts.tile([1, 128], FP32, tag="ones", name="ones_sb")
    nc.vector.memset(sb["ones"], 1.0)
    for key, src in [("noff", d["noff"]), ("coef", d["coef"]),
                     ("b1", d["b1c"]), ("b2", d["b2c"])]:
        nc.sync.dma_start(sb[key], src[:, :])
    w2f_sb = consts.tile([128, HC], FP32, tag="w2f", name="w2f_sb")
    nc.sync.dma_start(w2f_sb, d["w2c"][:, :])

    xr = [big.tile([128, BL], F32R, tag=f"xr{k}", name=f"xr{k}")
          for k in range(KC)]
    xh = [big.tile([128, BL], BF16, tag=f"xh{k}", name=f"xh{k}")
          for k in range(KC)]
    xl = [big.tile([128, BL], BF16, tag=f"xl{k}", name=f"xl{k}")
          for k in range(KC)]
    wr = [big.tile([128, H], F32R, tag=f"wr{k}", name=f"wr{k}")
          for k in range(KC)]
    wh = [big.tile([128, H], BF16, tag=f"wh{k}", name=f"wh{k}")
          for k in range(KC)]
    wl = [big.tile([128, H], BF16, tag=f"wl{k}", name=f"wl{k}")
          for k in range(KC)]

    # RBF features first (small feats DMAs must beat the big DMAs into the
    # queues; the in-order PE's first instruction waits on them)
    for c in range(NCOL):
        for n in range(NT):
            bsl = slice(n * 512, (n + 1) * 512)
            kk = KC_EMB + c
            fe = tmp.tile([1, 512], FP32, tag="fe")
            nc.sync.dma_start(
                fe, d["feats"][:, c * BL + n * 512 : c * BL + (n + 1) * 512])
            bc = pp.tile([128, 512], FP32, tag="ps", name=f"bc_{c}_{n}")
            nc.tensor.matmul(bc, lhsT=sb["ones"][:, :], rhs=fe,
                             start=True, stop=True)
            d2 = tmp.tile([128, 512], FP32, tag="d2")
            nc.scalar.activation(d2, bc, AF.Square,
                                 bias=sb["noff"][:, c : c + 1], scale=1.0)
            rb = tmp.tile([128, 512], FP32, tag="rb")
            nc.scalar.activation(rb, d2, AF.Exp,
                                 scale=sb["coef"][:, c : c + 1])
            nc.vector.tensor_copy(xr[kk][:, bsl], rb)       # round to f32r
            nc.vector.tensor_copy(xh[kk][:, bsl], rb)       # round to bf16
            back = tmp.tile([128, 512], FP32, tag="back")
            nc.vector.tensor_sub(back, rb, xr[kk][:, bsl].bitcast(FP32))
            nc.vector.tensor_copy(xl[kk][:, bsl], back)     # residual -> bf16

    # big DMAs (k-ascending so the first k-sweep streams) + derived bf16
    # "high" planes (DVE casts of the f32r planes; the 2^-12 difference vs
    # bf16(original) only enters the ~2^-13-scale correction terms)
    for k in range(KC):
        ksl = slice(k * 128, (k + 1) * 128)
        nc.sync.dma_start(wr[k][:, :], d["w1r"][ksl, :])
        nc.sync.dma_start(wl[k][:, :], d["w1lb"][ksl, :])
        if k < KC_EMB:
            nc.sync.dma_start(xr[k][:, :], d["ehr"][ksl, :])
            nc.sync.dma_start(xl[k][:, :], d["ehlb"][ksl, :])
            nc.vector.tensor_copy(xh[k][:, :], xr[k].bitcast(FP32))
        nc.vector.tensor_copy(wh[k][:, :], wr[k].bitcast(FP32))

    hs = [big.tile([128, 512], FP32, tag=f"h{m}", name=f"h{m}")
          for m in range(HC)]
    for n in range(NT):
        bsl = slice(n * 512, (n + 1) * 512)
        hsl = slice(0, 512)
        p2 = pp.tile([1, 512], FP32, tag="ps", name=f"p2_{n}")
        for g in range(2):
            ms = range(4 * g, 4 * g + 4)
            ps = {m: pp.tile([128, 512], FP32, tag="ps", name=f"ps_{n}_{g}_{m}")
                  for m in ms}
            for k in range(KC):
                for m in ms:
                    msl = slice(m * 128, (m + 1) * 128)
                    nc.tensor.matmul(ps[m], lhsT=wr[k][:, msl],
                                     rhs=xr[k][:, bsl],
                                     start=(k == 0), stop=False)
                    nc.tensor.matmul(ps[m], lhsT=wh[k][:, msl],
                                     rhs=xl[k][:, bsl],
                                     start=False, stop=False)
                    nc.tensor.matmul(ps[m], lhsT=wl[k][:, msl],
                                     rhs=xh[k][:, bsl],
                                     start=False, stop=(k == KC - 1))
            for m in ms:
                nc.scalar.activation(hs[m][:, hsl], ps[m], AF.Relu,
                                     bias=sb["b1"][:, m : m + 1], scale=1.0)
            for m in ms:
                nc.tensor.matmul(p2, lhsT=w2f_sb[:, m : m + 1],
                                 rhs=hs[m][:, hsl],
                                 start=(m == 0), stop=(m == HC - 1))
        o_sb = outp.tile([1, 512], FP32, tag="o")
        nc.vector.tensor_scalar_add(o_sb, p2, sb["b2"][:1, :1])
        nc.sync.dma_start(d["out"][:, bsl], o_sb)


def _build_bf16s(nc, tc, dram, pools, spec):
    """Single-pass bf16 L1 with W2 folded into W1 on the host.

    Layout: batch on PSUM partitions (lhsT = x chunk [K=128, 128 batch],
    rhs = W1' chunk [K=128, 512 hidden]).  Host pre-scales W1 column j by
    |w2_j| and permutes hidden units so all w2>=0 columns come first; the
    H->1 second layer then collapses to sign-weighted free-dim sums that
    ride the ReLU PSUM drains via accum_out -- zero PE cycles for layer 2.
    Drains alternate ScalarE/DVE so PSUM banks free at 2x rate.

    spec = (pieces, has_b1) with pieces = ((half, lo, hi, sign), ...):
    column ranges of each 512-wide hidden tile with uniform w2 sign.
    """
    big, consts, tmp, outp, pp = pools
    d = dram
    noff = consts.tile([R, NCOL], FP32, tag="noff", name="noff_sb")
    coef = consts.tile([R, NCOL], FP32, tag="coef", name="coef_sb")
    b2c = consts.tile([128, 1], FP32, tag="b2", name="b2_sb")
    ones = consts.tile([1, 128], F32R, tag="ones", name="ones_sb")
    feats = consts.tile([1, NCOL * BL], F32R, tag="feats", name="feats_sb")
    nc.vector.memset(ones, 1.0)
    pieces, has_b1 = spec

    wh = big.tile([128, KC * H], BF16, tag="wh", name="wh")
    xh = big.tile([128, KC * BL], BF16, tag="xh", name="xh")
    accs = outp.tile([128, 24], FP32, tag="accs", name="accs")
    ocol = outp.tile([128, 8], FP32, tag="ocol", name="ocol")
    if has_b1:
        b1r = consts.tile([1, H], BF16, tag="b1r", name="b1r_sb")
        onesb = consts.tile([1, 128], BF16, tag="onesb", name="onesb_sb")
        nc.vector.memset(onesb, 1.0)
        nc.sync.dma_start(b1r, d["b1r"][:, :])

    # DMA order: small RBF inputs first (PE's first work), then k-chunks
    # interleaved x/w in consumption order.
    nc.sync.dma_start(feats, d["feats"][:, :])
    nc.sync.dma_start(noff, d["noff"][:, :])
    nc.sync.dma_start(coef, d["coef"][:, :])
    nc.sync.dma_start(b2c, d["b2c"][:, :])
    for k in range(KC):
        ksl = slice(k * BL, (k + 1) * BL)
        if k < KC_EMB:
            nc.sync.dma_start(xh[:, ksl], d["ehk"][:, ksl])
        nc.sync.dma_start(wh[:, k * H : (k + 1) * H],
                          d["w1k"][:, k * H : (k + 1) * H])

    # RBF features: broadcast feats[c] over partitions via K=1 f32r matmul
    # (cheap PE p-state warmup), then (x+noff)^2 and exp on ScalarE.
    for c in range(NCOL):
        for n in range(NT):
            bc = pp.tile([128, 512], FP32, tag="ps", name=f"bc_{c}_{n}")
            nc.tensor.matmul(
                bc, lhsT=ones[:, :],
                rhs=feats[:, c * BL + n * 512 : c * BL + (n + 1) * 512],
                start=True, stop=True,
            )
            d2 = tmp.tile([128, 512], FP32, tag="d2")
            nc.scalar.activation(d2, bc, AF.Square,
                                 bias=noff[:, c : c + 1], scale=1.0)
            nc.scalar.activation(
                xh[:, (KC_EMB + c) * BL + n * 512 : (KC_EMB + c) * BL + (n + 1) * 512],
                d2, AF.Exp, scale=coef[:, c : c + 1])

    # L1: two phases of 4 batch-tiles; 8 PSUM accumulators per phase.
    drain_idx = 0
    for phase in range(2):
        bts = range(4 * phase, 4 * phase + 4)
        ps = {}
        for bt in bts:
            for hf in range(2):
                ps[bt, hf] = pp.tile([128, 512], FP32, tag="ps",
                                     name=f"ps_{bt}_{hf}")
        for k in range(KC):
            for bt in bts:
                for hf in range(2):
                    nc.tensor.matmul(
                        ps[bt, hf],
                        lhsT=xh[:, k * BL + bt * 128 : k * BL + (bt + 1) * 128],
                        rhs=wh[:, k * H + hf * 512 : k * H + (hf + 1) * 512],
                        start=(k == 0), stop=(k == KC - 1 and not has_b1),
                    )
        if has_b1:
            for bt in bts:
                for hf in range(2):
                    nc.tensor.matmul(
                        ps[bt, hf], lhsT=onesb[:, :],
                        rhs=b1r[:, hf * 512 : (hf + 1) * 512],
                        start=False, stop=True,
                    )
        # ReLU drains with free-dim accumulation = the whole second layer.
        # Alternate engines in bank-allocation order so the next phase's
        # PSUM WAR dependencies clear at two-drains-per-600ns.
        for bt in bts:
            for hf in range(2):
                pcs = [p for p in range(len(pieces)) if pieces[p][0] == hf]
                for p in pcs:
                    _, lo, hi, _sgn = pieces[p]
                    sc = tmp.tile([128, 512], BF16, tag="sc")
                    acc = accs[:, p * 8 + bt : p * 8 + bt + 1]
                    if drain_idx % 2 == 0:
                        nc.scalar.activation(sc[:, lo:hi], ps[bt, hf][:, lo:hi],
                                             AF.Relu, accum_out=acc)
                    else:
                        nc.vector.tensor_scalar(
                            sc[:, lo:hi], ps[bt, hf][:, lo:hi], 0.0, 0.0,
                            mybir.AluOpType.max, mybir.AluOpType.add,
                            accum_out=acc)
                drain_idx += 1

    # Combine piece sums: out[p, bt] = sum_p sign * acc + b2.
    npieces = len(pieces)
    pos = [p for p in range(npieces) if pieces[p][3] > 0]
    neg = [p for p in range(npieces) if pieces[p][3] < 0]

    def _pslice(p):
        return accs[:, p * 8 : p * 8 + 8]

    if pos:
        cur = _pslice(pos[0])
        for p in pos[1:]:
            nc.vector.tensor_tensor(ocol, cur, _pslice(p), mybir.AluOpType.add)
            cur = ocol
    else:
        nc.vector.memset(ocol, 0.0)
        cur = ocol
    for p in neg:
        nc.vector.tensor_tensor(ocol, cur, _pslice(p),
                                mybir.AluOpType.subtract)
        cur = ocol
    nc.vector.tensor_scalar_add(ocol, cur, b2c[:, :1])
    nc.sync.dma_start(d["out"][:, :], ocol)


def _build_fp32(nc, tc, dram, pools):
    big, consts, tmp, outp, pp = pools
    d = dram
    sb = _consts(nc, consts, dict(
        feats=d["feats"], noff=d["noff"], coef=d["coef"],
        b1=d["b1c"], b2=d["b2c"],
    ))
    w2_sb = consts.tile([128, HC], FP32, tag="w2")
    nc.sync.dma_start(w2_sb, d["w2c"][:, :])

    xt = [big.tile([128, BL], FP32, tag=f"xt{k}", name=f"xt{k}")
          for k in range(KC)]
    w1s = [big.tile([128, H], FP32, tag=f"w1_{k}", name=f"w1_{k}")
           for k in range(KC)]
    hs = [big.tile([128, BL], FP32, tag=f"h{m}", name=f"h{m}")
          for m in range(HC)]

    for k in range(KC):
        nc.sync.dma_start(w1s[k][:, :], d["w1"][k * 128 : (k + 1) * 128, :])
        if k < KC_EMB:
            nc.sync.dma_start(xt[k][:, :], d["embT"][k * 128 : (k + 1) * 128, :])

    for c in range(NCOL):
        for n in range(NT):
            bsl = slice(n * 512, (n + 1) * 512)
            bc = _rbf_psum(nc, pp, sb, c, n)
            d2 = tmp.tile([128, 512], FP32, tag="d2")
            nc.scalar.activation(d2, bc, AF.Square,
                                 bias=sb["noff"][:, c : c + 1], scale=1.0)
            nc.scalar.activation(xt[KC_EMB + c][:, bsl], d2, AF.Exp,
                                 scale=sb["coef"][:, c : c + 1])

    for n in range(NT):
        bsl = slice(n * 512, (n + 1) * 512)
        for g in range(2):
            ms = range(4 * g, 4 * g + 4)
            ps = {m: pp.tile([128, 512], FP32, tag="ps", name=f"ps_{n}_{g}_{m}")
                  for m in ms}
            for k in range(KC):
                for m in ms:
                    nc.tensor.matmul(
                        ps[m], lhsT=w1s[k][:, m * 128 : (m + 1) * 128],
                        rhs=xt[k][:, bsl],
                        start=(k == 0), stop=(k == KC - 1),
                    )
            for m in ms:
                nc.scalar.activation(hs[m][:, bsl], ps[m], AF.Relu,
                                     bias=sb["b1"][:, m : m + 1], scale=1.0)

    for n in range(NT):
        bsl = slice(n * 512, (n + 1) * 512)
        p2 = pp.tile([1, 512], FP32, tag="ps", name=f"p2_{n}")
        for m in range(HC):
            nc.tensor.matmul(p2, lhsT=w2_sb[:, m : m + 1], rhs=hs[m][:, bsl],
                             start=(m == 0), stop=(m == HC - 1))
        o_sb = outp.tile([1, 512], FP32, tag="o")
        nc.vector.tensor_scalar_add(o_sb, p2, sb["b2"][:1, :1])
        nc.sync.dma_start(d["out"][:, bsl], o_sb)


def _build_bf16x3(nc, tc, dram, pools):
    big, consts, tmp, outp, pp = pools
    d = dram
    sb = _consts(nc, consts, dict(
        feats=d["feats"], noff=d["noff"], coef=d["coef"],
        b1=d["b1c"], b2=d["b2c"],
    ))
    w2_sb = consts.tile([128, HC], FP32, tag="w2")
    nc.sync.dma_start(w2_sb, d["w2c"][:, :])

    xh = [big.tile([128, BL], BF16, tag=f"xh{k}", name=f"xh{k}")
          for k in range(KC)]
    xl = [big.tile([128, BL], BF16, tag=f"xl{k}", name=f"xl{k}")
          for k in range(KC)]
    wh = [big.tile([128, H], BF16, tag=f"wh{k}", name=f"wh{k}")
          for k in range(KC)]
    wl = [big.tile([128, H], BF16, tag=f"wl{k}", name=f"wl{k}")
          for k in range(KC)]
    hs = [big.tile([128, BL], FP32, tag=f"h{m}", name=f"h{m}")
          for m in range(HC)]

    for k in range(KC):
        ksl = slice(k * 128, (k + 1) * 128)
        nc.sync.dma_start(wh[k][:, :], d["w1h"][ksl, :])
        nc.sync.dma_start(wl[k][:, :], d["w1l"][ksl, :])
        if k < KC_EMB:
            nc.sync.dma_start(xh[k][:, :], d["ehT"][ksl, :])
            nc.sync.dma_start(xl[k][:, :], d["elT"][ksl, :])

    for c in range(NCOL):
        for n in range(NT):
            bsl = slice(n * 512, (n + 1) * 512)
            kk = KC_EMB + c
            bc = _rbf_psum(nc, pp, sb, c, n)
            d2 = tmp.tile([128, 512], FP32, tag="d2")
            nc.scalar.activation(d2, bc, AF.Square,
                                 bias=sb["noff"][:, c : c + 1], scale=1.0)
            rb = tmp.tile([128, 512], FP32, tag="rb")
            nc.scalar.activation(rb, d2, AF.Exp,
                                 scale=sb["coef"][:, c : c + 1])
            nc.vector.tensor_copy(xh[kk][:, bsl], rb)      # round to bf16
            back = tmp.tile([128, 512], FP32, tag="back")
            nc.vector.tensor_copy(back, xh[kk][:, bsl])    # widen
            nc.vector.tensor_sub(back, rb, back)           # residual
            nc.vector.tensor_copy(xl[kk][:, bsl], back)

    for n in range(NT):
        bsl = slice(n * 512, (n + 1) * 512)
        p2 = pp.tile([1, 512], FP32, tag="ps", name=f"p2_{n}")
        for g in range(2):
            ms = range(4 * g, 4 * g + 4)
            ps = {m: pp.tile([128, 512], FP32, tag="ps", name=f"ps_{n}_{g}_{m}")
                  for m in ms}
            for k in range(KC):
                for m in ms:
                    msl = slice(m * 128, (m + 1) * 128)
                    nc.tensor.matmul(ps[m], lhsT=wh[k][:, msl],
                                     rhs=xh[k][:, bsl],
                                     start=(k == 0), stop=False)
                    nc.tensor.matmul(ps[m], lhsT=wh[k][:, msl],
                                     rhs=xl[k][:, bsl],
                                     start=False, stop=False)
                    nc.tensor.matmul(ps[m], lhsT=wl[k][:, msl],
                                     rhs=xh[k][:, bsl],
                                     start=False, stop=(k == KC - 1))
            for m in ms:
                nc.scalar.activation(hs[m][:, bsl], ps[m], AF.Relu,
                                     bias=sb["b1"][:, m : m + 1], scale=1.0)
            for m in ms:
                nc.tensor.matmul(p2, lhsT=w2_sb[:, m : m + 1],
                                 rhs=hs[m][:, bsl],
                                 start=(m == 0), stop=(m == HC - 1))
        o_sb = outp.tile([1, 512], FP32, tag="o")
        nc.vector.tensor_scalar_add(o_sb, p2, sb["b2"][:1, :1])
        nc.sync.dma_start(d["out"][:, bsl], o_sb)



def _build_bf16x3b(nc, tc, dram, pools):
    """bf16x3 + bf16-split second matmul (saves the 4-cycles/row fp32 PE
    cost of the H->1 dot: 16x853ns -> 48x213ns + overlapped DVE splits)."""
    big, consts, tmp, outp, pp = pools
    d = dram
    sb = _consts(nc, consts, dict(
        feats=d["feats"], noff=d["noff"], coef=d["coef"],
        b1=d["b1c"], b2=d["b2c"],
    ))
    w2h_sb = consts.tile([128, HC], BF16, tag="w2h", name="w2h_sb")
    w2l_sb = consts.tile([128, HC], BF16, tag="w2l", name="w2l_sb")
    nc.sync.dma_start(w2h_sb, d["w2h"][:, :])
    nc.sync.dma_start(w2l_sb, d["w2l"][:, :])

    xh = [big.tile([128, BL], BF16, tag=f"xh{k}", name=f"xh{k}")
          for k in range(KC)]
    xl = [big.tile([128, BL], BF16, tag=f"xl{k}", name=f"xl{k}")
          for k in range(KC)]
    wh = [big.tile([128, H], BF16, tag=f"wh{k}", name=f"wh{k}")
          for k in range(KC)]
    wl = [big.tile([128, H], BF16, tag=f"wl{k}", name=f"wl{k}")
          for k in range(KC)]
    hs = [big.tile([128, BL], FP32, tag=f"h{m}", name=f"h{m}")
          for m in range(HC)]
    hh = [big.tile([128, BL], BF16, tag=f"hh{m}", name=f"hh{m}")
          for m in range(HC)]
    hl = [big.tile([128, BL], BF16, tag=f"hl{m}", name=f"hl{m}")
          for m in range(HC)]

    for k in range(KC):
        ksl = slice(k * 128, (k + 1) * 128)
        nc.sync.dma_start(wh[k][:, :], d["w1h"][ksl, :])
        nc.sync.dma_start(wl[k][:, :], d["w1l"][ksl, :])
        if k < KC_EMB:
            nc.sync.dma_start(xh[k][:, :], d["ehT"][ksl, :])
            nc.sync.dma_start(xl[k][:, :], d["elT"][ksl, :])

    for c in range(NCOL):
        for n in range(NT):
            bsl = slice(n * 512, (n + 1) * 512)
            kk = KC_EMB + c
            bc = _rbf_psum(nc, pp, sb, c, n)
            d2 = tmp.tile([128, 512], FP32, tag="d2")
            nc.scalar.activation(d2, bc, AF.Square,
                                 bias=sb["noff"][:, c : c + 1], scale=1.0)
            rb = tmp.tile([128, 512], FP32, tag="rb")
            nc.scalar.activation(rb, d2, AF.Exp,
                                 scale=sb["coef"][:, c : c + 1])
            nc.vector.tensor_copy(xh[kk][:, bsl], rb)
            back = tmp.tile([128, 512], FP32, tag="back")
            nc.vector.tensor_copy(back, xh[kk][:, bsl])
            nc.vector.tensor_sub(back, rb, back)
            nc.vector.tensor_copy(xl[kk][:, bsl], back)

    for n in range(NT):
        bsl = slice(n * 512, (n + 1) * 512)
        p2 = pp.tile([1, 512], FP32, tag="ps", name=f"p2_{n}")
        for g in range(2):
            ms = range(4 * g, 4 * g + 4)
            ps = {m: pp.tile([128, 512], FP32, tag="ps", name=f"ps_{n}_{g}_{m}")
                  for m in ms}
            for k in range(KC):
                for m in ms:
                    msl = slice(m * 128, (m + 1) * 128)
                    nc.tensor.matmul(ps[m], lhsT=wh[k][:, msl],
                                     rhs=xh[k][:, bsl],
                                     start=(k == 0), stop=False)
                    nc.tensor.matmul(ps[m], lhsT=wh[k][:, msl],
                                     rhs=xl[k][:, bsl],
                                     start=False, stop=False)
                    nc.tensor.matmul(ps[m], lhsT=wl[k][:, msl],
                                     rhs=xh[k][:, bsl],
                                     start=False, stop=(k == KC - 1))
            for m in ms:
                nc.scalar.activation(hs[m][:, bsl], ps[m], AF.Relu,
                                     bias=sb["b1"][:, m : m + 1], scale=1.0)
                nc.vector.tensor_copy(hh[m][:, bsl], hs[m][:, bsl])
                back2 = tmp.tile([128, 512], FP32, tag="back2")
                nc.vector.tensor_copy(back2, hh[m][:, bsl])
                nc.vector.tensor_sub(back2, hs[m][:, bsl], back2)
                nc.vector.tensor_copy(hl[m][:, bsl], back2)
            for m in ms:
                mm = slice(m, m + 1)
                nc.tensor.matmul(p2, lhsT=w2h_sb[:, mm], rhs=hh[m][:, bsl],
                                 start=(m == 0), stop=False)
                nc.tensor.matmul(p2, lhsT=w2h_sb[:, mm], rhs=hl[m][:, bsl],
                                 start=False, stop=False)
                nc.tensor.matmul(p2, lhsT=w2l_sb[:, mm], rhs=hh[m][:, bsl],
                                 start=False, stop=(n >= 0 and m == HC - 1))
        o_sb = outp.tile([1, 512], FP32, tag="o")
        nc.vector.tensor_scalar_add(o_sb, p2, sb["b2"][:1, :1])
        nc.sync.dma_start(d["out"][:, bsl], o_sb)


def _build_nc(mode: str, spec=None) -> bass.Bass:
    # Bacc (not raw Bass): its finalize() runs move_matmul_waits_to_ldweights
    # + generate_event_semaphores, which split semaphore waits that exceed
    # the per-instruction hardware limit (walrus otherwise fails codegen).
    nc = bacc.Bacc()

    if mode == "bf16s":
        if spec is None:
            spec = (((0, 0, 512, 1), (1, 0, 512, -1)), False)
        d = {}
        d["feats"] = nc.dram_tensor("feats", [1, NCOL * BL], F32R,
                                    kind="ExternalInput")
        d["noff"] = nc.dram_tensor("noff", [R, NCOL], FP32,
                                   kind="ExternalInput")
        d["coef"] = nc.dram_tensor("coef", [R, NCOL], FP32,
                                   kind="ExternalInput")
        d["b2c"] = nc.dram_tensor("b2c", [128, 1], FP32, kind="ExternalInput")
        d["w1k"] = nc.dram_tensor("w1k", [128, KC * H], BF16,
                                  kind="ExternalInput")
        d["ehk"] = nc.dram_tensor("ehk", [128, KC_EMB * BL], BF16,
                                  kind="ExternalInput")
        if spec[1]:
            d["b1r"] = nc.dram_tensor("b1r", [1, H], BF16,
                                      kind="ExternalInput")
        d["out"] = nc.dram_tensor("out", [128, 8], FP32,
                                  kind="ExternalOutput")
        with tile.TileContext(nc) as tc:
            with (
                tc.tile_pool(name="big", bufs=1) as big,
                tc.tile_pool(name="consts", bufs=1) as consts,
                tc.tile_pool(name="tmp", bufs=4) as tmp,
                tc.tile_pool(name="outp", bufs=1) as outp,
                tc.tile_pool(name="psum", bufs=8, space="PSUM") as pp,
            ):
                _build_bf16s(nc, tc, d, (big, consts, tmp, outp, pp), spec)
        nc.finalize()
        return nc

    d = {}
    d["feats"] = nc.dram_tensor("feats", [1, NCOL * BL], FP32,
                                kind="ExternalInput")
    d["b1c"] = nc.dram_tensor("b1c", [128, HC], FP32, kind="ExternalInput")
    d["w2c"] = nc.dram_tensor("w2c", [128, HC], FP32, kind="ExternalInput")
    d["b2c"] = nc.dram_tensor("b2c", [1, 1], FP32, kind="ExternalInput")
    d["noff"] = nc.dram_tensor("noff", [R, NCOL], FP32, kind="ExternalInput")
    d["coef"] = nc.dram_tensor("coef", [R, NCOL], FP32, kind="ExternalInput")
    d["out"] = nc.dram_tensor("out", [1, BL], FP32, kind="ExternalOutput")

    if mode == "fp32":
        d["embT"] = nc.dram_tensor("embT", [D, BL], FP32, kind="ExternalInput")
        d["w1"] = nc.dram_tensor("w1", [KTOT, H], FP32, kind="ExternalInput")
    elif mode in ("bf16x3", "bf16x3b"):
        for n2 in ("ehT", "elT"):
            d[n2] = nc.dram_tensor(n2, [D, BL], BF16, kind="ExternalInput")
        for n2 in ("w1h", "w1l"):
            d[n2] = nc.dram_tensor(n2, [KTOT, H], BF16, kind="ExternalInput")
        if mode == "bf16x3b":
            d["w2h"] = nc.dram_tensor("w2h", [128, HC], BF16,
                                      kind="ExternalInput")
            d["w2l"] = nc.dram_tensor("w2l", [128, HC], BF16,
                                      kind="ExternalInput")
    elif mode in ("f32r1", "f32r3"):
        d["ehr"] = nc.dram_tensor("ehr", [D, BL], F32R, kind="ExternalInput")
        d["w1r"] = nc.dram_tensor("w1r", [KTOT, H], F32R, kind="ExternalInput")
        d["w2r"] = nc.dram_tensor("w2r", [128, HC], F32R, kind="ExternalInput")
        if mode == "f32r3":
            d["ehl"] = nc.dram_tensor("ehl", [D, BL], F32R,
                                      kind="ExternalInput")
            d["w1l"] = nc.dram_tensor("w1l", [KTOT, H], F32R,
                                      kind="ExternalInput")
            d["w2l"] = nc.dram_tensor("w2l", [128, HC], F32R,
                                      kind="ExternalInput")
    elif mode == "f32rh":
        d["ehr"] = nc.dram_tensor("ehr", [D, BL], F32R, kind="ExternalInput")
        d["ehlb"] = nc.dram_tensor("ehlb", [D, BL], BF16, kind="ExternalInput")
        d["w1r"] = nc.dram_tensor("w1r", [KTOT, H], F32R, kind="ExternalInput")
        d["w1lb"] = nc.dram_tensor("w1lb", [KTOT, H], BF16,
                                   kind="ExternalInput")
    else:
        raise ValueError(mode)

    with tile.TileContext(nc) as tc:
        with (
            tc.tile_pool(name="big", bufs=1) as big,
            tc.tile_pool(name="consts", bufs=1) as consts,
            tc.tile_pool(name="tmp", bufs=3) as tmp,
            tc.tile_pool(name="outp", bufs=2) as outp,
            tc.tile_pool(name="psum", bufs=8, space="PSUM") as pp,
        ):
            pools = (big, consts, tmp, outp, pp)
            if mode == "fp32":
                _build_fp32(nc, tc, d, pools)
            elif mode == "bf16x3":
                _build_bf16x3(nc, tc, d, pools)
            elif mode == "bf16x3b":
                _build_bf16x3b(nc, tc, d, pools)
            elif mode == "f32rh":
                _build_f32rh(nc, tc, d, pools)
            else:
                _build_f32r(nc, tc, d, pools, three_pass=(mode == "f32r3"))

    # run Bacc's compile pipeline (wait splitting, register allocation);
    # run_bass_via_pjrt serializes nc.m as-is and never finalizes.
    nc.finalize()
    return nc


def _bf16_pair(a: np.ndarray):
    """Split fp32 array into (hi, lo) bf16 arrays with hi+lo ~ a."""
    import ml_dtypes

    hi = a.astype(ml_dtypes.bfloat16)
    lo = (a - hi.astype(np.float32)).astype(ml_dtypes.bfloat16)
    return hi, lo


def _round_f32r(a: np.ndarray) -> np.ndarray:
    """Round fp32 to f32r (11-bit mantissa, round-half-up at bit 12) --
    bit-exact with the hardware's cast (verified against gpsimd cast-DMA)."""
    v = np.ascontiguousarray(a, dtype=np.float32).view(np.uint32)
    r = (((v.astype(np.uint64) + (1 << 11)) >> 12) << 12).astype(np.uint32)
    return r.view(np.float32)


def _f32r_pair(a: np.ndarray):
    hi = _round_f32r(a)
    lo = _round_f32r(a - hi)
    return hi, lo


_NC_CACHE: dict = {}


def _kernel_bf16s(emb, feats, rbf_offset, rbf_coeff, W1, b1, W2, b2,
                  component_idx):
    import ml_dtypes

    w2 = np.asarray(W2, np.float32)[0, :, 0]                     # [1024]
    b1v = np.asarray(b1, np.float32)[0] * np.abs(w2)             # folded
    sgn = np.where(w2 >= 0, 1, -1)
    perm = np.argsort(-sgn, kind="stable")                       # positives first
    npos = int((sgn > 0).sum())
    w1s = np.asarray(W1, np.float32)[0] * np.abs(w2)[None, :]    # [1280, 1024]
    w1p = w1s[:, perm]
    b1p = b1v[perm]

    pieces = []
    for hf in range(2):
        lo_g, hi_g = hf * 512, (hf + 1) * 512
        if npos <= lo_g:
            pieces.append((hf, 0, 512, -1))
        elif npos >= hi_g:
            pieces.append((hf, 0, 512, 1))
        else:
            pieces.append((hf, 0, npos - lo_g, 1))
            pieces.append((hf, npos - lo_g, 512, -1))
    has_b1 = bool(np.any(b1p != 0.0))
    spec = (tuple(pieces), has_b1)

    w1k = np.ascontiguousarray(
        w1p.reshape(KC, 128, H).transpose(1, 0, 2).reshape(128, KC * H)
    ).astype(ml_dtypes.bfloat16)                                 # [128, 10240]
    b2v = np.asarray(b2, np.float32)[0, 0]
    shared = dict(
        w1k=w1k,
        b2c=np.full((128, 1), b2v, np.float32),
        noff=np.ascontiguousarray(-np.asarray(rbf_offset, np.float32).T),
        coef=np.ascontiguousarray(np.broadcast_to(
            np.asarray(rbf_coeff, np.float32)[None, :], (R, NCOL))),
    )
    if has_b1:
        shared["b1r"] = b1p.reshape(1, H).astype(ml_dtypes.bfloat16)

    feats = np.ascontiguousarray(np.asarray(feats, np.float32))
    emb = np.ascontiguousarray(np.asarray(emb, np.float32))
    in_maps = []
    for i in range(NCORES):
        s = slice(i * BL, (i + 1) * BL)
        embT = emb[s].T                                          # [768, 1024]
        ehk = np.ascontiguousarray(
            embT.reshape(KC_EMB, 128, BL).transpose(1, 0, 2)
            .reshape(128, KC_EMB * BL)
        ).astype(ml_dtypes.bfloat16)
        m = dict(
            feats=_round_f32r(
                np.ascontiguousarray(feats[:, s]).reshape(1, NCOL * BL)),
            ehk=ehk,
            **shared,
        )
        in_maps.append(m)

    key = ("bf16s", spec)
    if key not in _NC_CACHE:
        _NC_CACHE[key] = _build_nc("bf16s", spec)

    res = run_bass_kernel_spmd(_NC_CACHE[key], in_maps, list(range(NCORES)))

    # out[p, bt] = batch row bt*128 + p of the core's 1024-row slice
    pred = np.concatenate(
        [res.results[i]["out"].T.reshape(BL) for i in range(NCORES)]
    )                                                            # [8192]
    component_idx = np.asarray(component_idx)
    order = np.argsort(component_idx, kind="stable")
    inv = np.argsort(order, kind="stable")
    return pred[inv].reshape(B, 1).astype(np.float32)


def kernel(emb, feats, rbf_offset, rbf_coeff, W1, b1, W2, b2, component_idx):
    mode = os.environ.get("KERNEL_MODE", "bf16s")
    if mode == "bf16s":
        return _kernel_bf16s(emb, feats, rbf_offset, rbf_coeff, W1, b1, W2,
                             b2, component_idx)
    emb = np.ascontiguousarray(emb, dtype=np.float32)
    feats = np.ascontiguousarray(feats, dtype=np.float32)
    rbf_offset = np.asarray(rbf_offset, dtype=np.float32)
    rbf_coeff = np.asarray(rbf_coeff, dtype=np.float32)
    W1 = np.asarray(W1, dtype=np.float32)
    b1 = np.asarray(b1, dtype=np.float32)
    W2 = np.asarray(W2, dtype=np.float32)
    b2 = np.asarray(b2, dtype=np.float32)
    component_idx = np.asarray(component_idx)

    # shared (expert-0 only) tensors
    w1_full = np.ascontiguousarray(W1[0])                        # [1280, 1024]
    w2c = np.ascontiguousarray(W2[0, :, 0].reshape(HC, 128).T)   # [128, 8]
    shared = dict(
        b1c=np.ascontiguousarray(b1[0].reshape(HC, 128).T),      # [128, 8]
        w2c=w2c,
        b2c=b2[0].reshape(1, 1),
        noff=np.ascontiguousarray(-rbf_offset.T),                # [128, 4]
        coef=np.ascontiguousarray(
            np.broadcast_to(rbf_coeff[None, :], (R, NCOL))),     # [128, 4]
    )
    if mode == "fp32":
        shared["w1"] = w1_full
    elif mode in ("bf16x3", "bf16x3b"):
        shared["w1h"], shared["w1l"] = _bf16_pair(w1_full)
        if mode == "bf16x3b":
            shared["w2h"], shared["w2l"] = _bf16_pair(w2c)
    elif mode == "f32rh":
        import ml_dtypes

        shared["w1r"] = _round_f32r(w1_full)
        shared["w1lb"] = (w1_full - shared["w1r"]).astype(ml_dtypes.bfloat16)
    else:
        shared["w1r"], w1l = _f32r_pair(w1_full)
        w2r, w2l = _f32r_pair(w2c)
        shared["w2r"] = w2r
        if mode == "f32r3":
            shared["w1l"] = w1l
            shared["w2l"] = w2l

    in_maps = []
    for i in range(NCORES):
        s = slice(i * BL, (i + 1) * BL)
        m = dict(
            feats=np.ascontiguousarray(feats[:, s]).reshape(1, NCOL * BL),
            **shared,
        )
        embT = np.ascontiguousarray(emb[s].T)                    # [768, 1024]
        if mode == "fp32":
            m["embT"] = embT
        elif mode in ("bf16x3", "bf16x3b"):
            m["ehT"], m["elT"] = _bf16_pair(embT)
        elif mode == "f32rh":
            import ml_dtypes

            m["ehr"] = _round_f32r(embT)
            m["ehlb"] = (embT - m["ehr"]).astype(ml_dtypes.bfloat16)
        else:
            m["ehr"], ehl = _f32r_pair(embT)
            if mode == "f32r3":
                m["ehl"] = ehl
        in_maps.append(m)

    if mode not in _NC_CACHE:
        _NC_CACHE[mode] = _build_nc(mode)

    res = run_bass_kernel_spmd(_NC_CACHE[mode], in_maps, list(range(NCORES)))

    pred = np.concatenate(
        [res.results[i]["out"].reshape(BL) for i in range(NCORES)]
    )                                                            # [8192]

    order = np.argsort(component_idx, kind="stable")
    inv = np.argsort(order, kind="stable")
    return pred[inv].reshape(B, 1).astype(np.float32)

